# revision 1
# baseline (speedup 1.0000x reference)
"""GTN (Graph Transformer Network) kernel on 8 TRN2 NeuronCores via Bass/Tile.

Problem nn_GTN_17162689314910:
  A: [E=5, N=2048, N] f32, X: [N, 256] f32, conv_w_*: [C=2, E, 1, 1] f32,
  gcn_weight: [256, 64] f32 -> out [N, C*64] f32.

Math (per channel c):
  a = sum_e softmax(w1)[c,e] A[e];  b, a1 likewise with w2, w3
  H0 = a @ b
  H0n = H0 * 1/(colsum(H0)+eps)          (norm add=False; diag term dropped,
                                          verified 3.8e-4 rel err in fp64)
  H1 = H0n @ a1
  H1d = H1 with diag set to 1            (norm add=True diag handling is
                                          mandatory: without it 4.1e-2)
  out_c = relu(H1d^T @ (X @ W) * 1/(colsum(H1d)+eps)[:,None])

Sharding: channel-split. Cores 0-3 = channel 0, cores 4-7 = channel 1;
within a group, 512-row shards (core c: rows 512*(c%4)...). bf16 compute,
fp32 PSUM. Collectives:
  C1: four 8-core Shared-output AllGather chunks carrying b-shard halves
      (+ colsum(a) partials) and a1-shard halves.
  C2: one 4-core-group AllReduce of readout partials [2048, 65]
      (64 feature cols + colsum(H1d) in col 64).
"""
import sys
import types

import numpy as np
import ml_dtypes

P = 128
N = 2048
S = 512            # shard rows per core
E = 5
TK = N // P        # 16 k tiles
TI = S // P        # 4 i tiles
Q = 512            # mm column-quarter width
EPS = 1e-8
GROUPS8 = [[0, 1, 2, 3, 4, 5, 6, 7]]
GROUPS4 = [[0, 1, 2, 3], [4, 5, 6, 7]]

_nc_cache = None


def _install_ntff_hook():
    if "antenv.axon_hooks" in sys.modules:
        return
    try:
        from trn_agent_boot.trn_boot import _ntff_profile_via_ctypes
        hook = _ntff_profile_via_ctypes("/opt/axon/libaxon_pjrt.so")
    except Exception:
        hook = None
    mod = types.ModuleType("antenv.axon_hooks")
    mod.get_axon_ntff_profile_hook = lambda: hook
    mod.set_axon_ntff_profile_hook = lambda h: None
    sys.modules["antenv.axon_hooks"] = mod


def _build_nc():
    import concourse.mybir as mybir
    import concourse.tile as tile
    from concourse import bacc
    from concourse.bass import ds
    from concourse.masks import make_identity

    bf16 = mybir.dt.bfloat16
    f32 = mybir.dt.float32
    Alu = mybir.AluOpType

    nc = bacc.Bacc(None)
    nc.num_devices = 8

    a_rows = nc.dram_tensor("a_rows", [E, S, N], bf16, kind="ExternalInput")
    s1 = nc.dram_tensor("s1", [P, E], f32, kind="ExternalInput")
    s2 = nc.dram_tensor("s2", [P, E], f32, kind="ExternalInput")
    s3 = nc.dram_tensor("s3", [P, E], f32, kind="ExternalInput")
    xt = nc.dram_tensor("xt", [256, S], bf16, kind="ExternalInput")
    w_in = nc.dram_tensor("w", [256, 64], bf16, kind="ExternalInput")
    doff = nc.dram_tensor("doff", [P, TI], f32, kind="ExternalInput")
    out = nc.dram_tensor("out", [N, 64], f32, kind="ExternalOutput")

    with tile.TileContext(nc) as tc:
        with (
            tc.tile_pool(name="pers", bufs=1) as pers,
            tc.tile_pool(name="work", bufs=3) as work,
            tc.tile_pool(name="pan", bufs=6) as panp,
            tc.tile_pool(name="ps", bufs=8, space="PSUM") as psp,
            tc.tile_pool(name="dram", bufs=1, space="DRAM") as dram,
        ):
            pid = nc.partition_id()
            g4_514 = (pid // 4) * (4 * 514)   # C1a block base (514 rows/rank)
            g4_512 = (pid // 4) * (4 * 512)   # C1b/c/d block base

            # ---- small SBUF constants ----
            s1_sb = pers.tile([P, E], f32, name="s1_sb")
            s2_sb = pers.tile([P, E], f32, name="s2_sb")
            s3_sb = pers.tile([P, E], f32, name="s3_sb")
            nc.sync.dma_start(s1_sb[:], s1[:])
            nc.sync.dma_start(s2_sb[:], s2[:])
            nc.sync.dma_start(s3_sb[:], s3[:])
            doff_sb = pers.tile([P, TI], f32, name="doff_sb")
            nc.sync.dma_start(doff_sb[:], doff[:])
            ident = pers.tile([P, P], bf16, name="ident")
            make_identity(nc, ident)
            ones_col = pers.tile([P, 1], bf16, name="ones_col")
            nc.gpsimd.memset(ones_col[:], 1.0)

            # ---- C1 AllGather buffers (8-core, Shared outputs) ----
            c1a_in = dram.tile([514, 1024], bf16, name="c1a_in")
            c1b_in = dram.tile([512, 1024], bf16, name="c1b_in")
            c1c_in = dram.tile([512, 1024], bf16, name="c1c_in")
            c1d_in = dram.tile([512, 1024], bf16, name="c1d_in")
            c1a_out = dram.tile([8 * 514, 1024], bf16, name="c1a_out",
                                addr_space="Shared")
            c1b_out = dram.tile([8 * 512, 1024], bf16, name="c1b_out",
                                addr_space="Shared")
            c1c_out = dram.tile([8 * 512, 1024], bf16, name="c1c_out",
                                addr_space="Shared")
            c1d_out = dram.tile([8 * 512, 1024], bf16, name="c1d_out",
                                addr_space="Shared")

            def ag(inp, outp):
                nc.gpsimd.collective_compute(
                    "AllGather", Alu.bypass, replica_groups=GROUPS8,
                    ins=[inp.opt()], outs=[outp.opt()])

            # ---- pass 1: conv b per row-tile, C1a/C1b launch ASAP;
            #      pass 2: conv a (+ca partials, aT transposes) -> C1 riders;
            #      pass 3: conv a1 -> C1c/C1d. A rows re-streamed per pass so
            #      the b-shard AllGather starts ~100us earlier. Conv tiles are
            #      split DVE/GpSimd (independent accumulation chains). ----
            aT = [pers.tile([P, S], bf16, name=f"aT_{k}") for k in range(TK)]
            ca_sb = pers.tile([1, N], bf16, name="ca_sb")
            ca_ps = [psp.tile([1, Q], f32, name=f"ca_ps_{cb}", tag="ps")
                     for cb in range(4)]

            def conv_tile(eng, dst, s_ap, Ats):
                eng.tensor_scalar(
                    dst[:], Ats[0][:], s_ap[:, 0:1], None, op0=Alu.mult)
                for e in range(1, E):
                    eng.scalar_tensor_tensor(
                        dst[:], Ats[e][:], s_ap[:, e:e + 1], dst[:],
                        op0=Alu.mult, op1=Alu.add)

            with tc.tile_pool(name="apool", bufs=2) as apool:
                def a_pass(body):
                    for t in range(TI):
                        Ats = [apool.tile([P, N], bf16, name=f"At{e}",
                                          tag=f"At{e}") for e in range(E)]
                        for e in range(E):
                            nc.sync.dma_start(Ats[e][:],
                                              a_rows[e, P * t:P * (t + 1), :])
                        eng = nc.vector
                        body(t, Ats, eng)

                def pass_b(t, Ats, eng):
                    bt = apool.tile([P, N], bf16, name="bt", tag="bt")
                    conv_tile(eng, bt, s2_sb, Ats)
                    nc.sync.dma_start(c1a_in[P * t:P * (t + 1), :], bt[:, 0:1024])
                    nc.sync.dma_start(c1b_in[P * t:P * (t + 1), :],
                                      bt[:, 1024:2048])
                a_pass(pass_b)
                ag(c1b_in, c1b_out)

                def pass_a1(t, Ats, eng):
                    a1t = apool.tile([P, N], bf16, name="a1t", tag="a1t")
                    conv_tile(eng, a1t, s3_sb, Ats)
                    nc.sync.dma_start(c1c_in[P * t:P * (t + 1), :], a1t[:, 0:1024])
                    nc.sync.dma_start(c1d_in[P * t:P * (t + 1), :],
                                      a1t[:, 1024:2048])
                a_pass(pass_a1)
                ag(c1c_in, c1c_out)

                def pass_a(t, Ats, eng):
                    at = apool.tile([P, N], bf16, name="at", tag="at")
                    conv_tile(eng, at, s1_sb, Ats)
                    for cb in range(4):
                        nc.tensor.matmul(
                            ca_ps[cb][:], ones_col[:], at[:, Q * cb:Q * (cb + 1)],
                            start=(t == 0), stop=(t == TI - 1))
                    for k in range(TK):
                        pt = psp.tile([P, P], bf16, name="ptt", tag="ps")
                        nc.tensor.transpose(pt[:], at[:, P * k:P * (k + 1)],
                                            ident[:])
                        nc.vector.tensor_copy(aT[k][:, P * t:P * (t + 1)], pt[:])
                a_pass(pass_a)
                for cb in range(4):
                    nc.vector.tensor_copy(ca_sb[0:1, Q * cb:Q * (cb + 1)],
                                          ca_ps[cb][:])
                nc.sync.dma_start(c1a_in[512:513, :], ca_sb[0:1, 0:1024])
                nc.sync.dma_start(c1a_in[513:514, :], ca_sb[0:1, 1024:2048])
                ag(c1a_in, c1a_out)
                ag(c1d_in, c1d_out)

            # ---- during C1: X@W, iota masks ----
            xt_sb = [pers.tile([P, S], bf16, name=f"xt_{k}") for k in range(2)]
            w_sb = [pers.tile([P, 64], bf16, name=f"w_{k}") for k in range(2)]
            for k in range(2):
                nc.sync.dma_start(xt_sb[k][:], xt[P * k:P * (k + 1), :])
                nc.sync.dma_start(w_sb[k][:], w_in[P * k:P * (k + 1), :])
            xwo = [pers.tile([P, 65], bf16, name=f"xwo_{t}") for t in range(TI)]
            for t in range(TI):
                px = psp.tile([P, 64], f32, name="px", tag="ps")
                for k in range(2):
                    nc.tensor.matmul(px[:], xt_sb[k][:, P * t:P * (t + 1)], w_sb[k][:],
                                     start=(k == 0), stop=(k == 1))
                nc.vector.tensor_copy(xwo[t][:, 0:64], px[:])
                nc.gpsimd.memset(xwo[t][:, 64:65], 1.0)

            u8 = mybir.dt.uint8
            masks = [pers.tile([P, N], u8, name=f"mask_{t}") for t in range(TI)]
            ones_t = pers.tile([P, N], bf16, name="ones_t")
            nc.gpsimd.memset(ones_t[:], 1.0)
            with tc.tile_pool(name="iotap", bufs=1) as iotap:
                iota_f = iotap.tile([P, N], f32, name="iota_f")
                nc.gpsimd.iota(iota_f[:], pattern=[[1, N]], base=0,
                               channel_multiplier=-1,
                               allow_small_or_imprecise_dtypes=True)
                for t in range(TI):
                    nc.vector.tensor_scalar(
                        masks[t][:], iota_f[:], doff_sb[:, t:t + 1], None,
                        op0=Alu.is_equal)

            # ---- ca_full: sum own group's 4 partials from c1a_out ----
            ca_full = work.tile([1, N], f32, name="ca_full", bufs=1)
            cp = []
            for r in range(4):
                off = g4_514 + r * 514 + 512
                cpr = work.tile([1, N], bf16, name="cpr", tag="cpr", bufs=2)
                nc.sync.dma_start(cpr[0:1, 0:1024], c1a_out[ds(off, 1), :])
                nc.sync.dma_start(cpr[0:1, 1024:N], c1a_out[ds(off + 1, 1), :])
                cp.append(cpr)
                if r == 1:
                    nc.vector.tensor_add(ca_full[:], cp[0][:], cp[1][:])
                elif r > 1:
                    nc.vector.tensor_add(ca_full[:], ca_full[:], cpr[:])
            ca_d = dram.tile([1, N], f32, name="ca_d")
            nc.sync.dma_start(ca_d[:], ca_full[:])
            caTb = work.tile([P, TK], bf16, name="caTb", bufs=1)
            nc.gpsimd.dma_start(caTb[:],
                                ca_d[0:1, :].rearrange("a (t p) -> (a p) t", p=P))

            # ---- mm1: H0[R,:] = a[R,:] @ b  (+ deg0 = ca @ b) ----
            H0 = [pers.tile([P, N], bf16, name=f"H0_{t}") for t in range(TI)]
            deg0 = work.tile([1, N], f32, name="deg0", bufs=1)
            for q in (2, 3, 0, 1):
                half_out = c1a_out if q < 2 else c1b_out
                col0 = (q % 2) * 512
                shard_rows = 514 if q < 2 else 512
                base = g4_514 if q < 2 else g4_512
                pts = [psp.tile([P, Q], f32, name=f"pt1_{i}", tag="ps")
                       for i in range(TI)]
                for k in range(TK):
                    pan = panp.tile([P, Q], bf16, name="pan")
                    off = base + (k // 4) * shard_rows + (k % 4) * P
                    nc.sync.dma_start(pan[:], half_out[ds(off, P), col0:col0 + Q])
                    for i in range(TI):
                        nc.tensor.matmul(pts[i][:], aT[k][:, P * i:P * (i + 1)],
                                         pan[:], start=(k == 0), stop=(k == TK - 1))
                for i in range(TI):
                    nc.vector.tensor_copy(H0[i][:, Q * q:Q * (q + 1)], pts[i][:])

            # ---- deg0 = ca @ b: dedicated panel pass (decoupled from mm1) ----
            ptds = [psp.tile([1, Q], f32, name=f"ptd_{qq}", tag="ps")
                    for qq in range(4)]
            for k in range(TK):
                pand = panp.tile([P, N], bf16, name="pand", tag="pand", bufs=3)
                offa = g4_514 + (k // 4) * 514 + (k % 4) * P
                offb = g4_512 + (k // 4) * 512 + (k % 4) * P
                nc.sync.dma_start(pand[:, 0:1024], c1a_out[ds(offa, P), :])
                nc.sync.dma_start(pand[:, 1024:2048], c1b_out[ds(offb, P), :])
                for qq in range(4):
                    nc.tensor.matmul(ptds[qq][:], caTb[:, k:k + 1],
                                     pand[:, Q * qq:Q * (qq + 1)],
                                     start=(k == 0), stop=(k == TK - 1))
            for qq in range(4):
                nc.vector.tensor_copy(deg0[0:1, Q * qq:Q * (qq + 1)], ptds[qq][:])

            # deginv0, bounced to per-partition [P, TK] layout
            nc.vector.tensor_scalar(deg0[:], deg0[:], float(EPS), None, op0=Alu.add)
            nc.vector.reciprocal(deg0[:], deg0[:])
            dinv0_d = dram.tile([1, N], f32, name="dinv0_d")
            nc.sync.dma_start(dinv0_d[:], deg0[:])
            dinv0T = work.tile([P, TK], f32, name="dinv0T", bufs=1)
            nc.sync.dma_start(dinv0T[:],
                              dinv0_d[0:1, :].rearrange("a (t p) -> (a p) t", p=P))

            # ---- lhsT for mm2: (H0^T) * deginv0[k], bf16 ----
            l0 = [pers.tile([P, S], bf16, name=f"l0_{k}") for k in range(TK)]
            for k in range(TK):
                for t in range(TI):
                    pt = psp.tile([P, P], bf16, name="ptt2", tag="ps")
                    nc.tensor.transpose(pt[:], H0[t][:, P * k:P * (k + 1)], ident[:])
                    nc.vector.tensor_scalar(
                        l0[k][:, P * t:P * (t + 1)], pt[:], dinv0T[:, k:k + 1], None,
                        op0=Alu.mult)

            # ---- mm2: H1[R,:] = H0n[R,:] @ a1 ----
            H1 = [pers.tile([P, N], bf16, name=f"H1_{t}") for t in range(TI)]
            for q in range(4):
                half_out = c1c_out if q < 2 else c1d_out
                col0 = (q % 2) * 512
                pts = [psp.tile([P, Q], f32, name=f"pt2_{i}", tag="ps")
                       for i in range(TI)]
                for k in range(TK):
                    pan = panp.tile([P, Q], bf16, name="pan2")
                    off = g4_512 + (k // 4) * 512 + (k % 4) * P
                    nc.sync.dma_start(pan[:], half_out[ds(off, P), col0:col0 + Q])
                    for i in range(TI):
                        nc.tensor.matmul(pts[i][:], l0[k][:, P * i:P * (i + 1)],
                                         pan[:], start=(k == 0), stop=(k == TK - 1))
                for i in range(TI):
                    nc.vector.tensor_copy(H1[i][:, Q * q:Q * (q + 1)], pts[i][:])

            # ---- norm1 diag: H1[i, 512r+128t+i] <- 1.0 ----
            for t in range(TI):
                nc.vector.copy_predicated(H1[t][:], masks[t][:], ones_t[:])

            # ---- readout partials + colsum(H1d): [N, 65] ----
            c2_in = dram.tile([N, 65], f32, name="c2_in")
            c2_out = dram.tile([N, 65], f32, name="c2_out")
            for j in range(TK):
                pr = psp.tile([P, 65], f32, name="pr", tag="ps")
                for i in range(TI):
                    nc.tensor.matmul(pr[:], H1[i][:, P * j:P * (j + 1)], xwo[i][:],
                                     start=(i == 0), stop=(i == TI - 1))
                ro = work.tile([P, 65], f32, name="ro")
                nc.vector.tensor_copy(ro[:], pr[:])
                nc.sync.dma_start(c2_in[P * j:P * (j + 1), :], ro[:])
            nc.gpsimd.collective_compute(
                "AllReduce", Alu.add, replica_groups=GROUPS4,
                ins=[c2_in.opt()], outs=[c2_out.opt()])

            # ---- final: relu(partial * deginv1) ----
            fo = work.tile([P, TK * 65], f32, name="fo", bufs=1)
            for j in range(TK):
                nc.sync.dma_start(fo[:, j * 65:(j + 1) * 65],
                                  c2_out[P * j:P * (j + 1), :])
            dinv1 = work.tile([P, TK], f32, name="dinv1", bufs=1)
            nc.vector.tensor_scalar(
                dinv1[:], fo[:, 64::65], float(EPS), None, op0=Alu.add)
            nc.vector.reciprocal(dinv1[:], dinv1[:])
            for j in range(TK):
                oj = work.tile([P, 64], f32, name="oj")
                nc.vector.tensor_scalar(oj[:], fo[:, j * 65:j * 65 + 64],
                                        dinv1[:, j:j + 1], 0.0,
                                        op0=Alu.mult, op1=Alu.max)
                nc.sync.dma_start(out[P * j:P * (j + 1), :], oj[:])

    nc.finalize()
    return nc


def _get_nc():
    global _nc_cache
    if _nc_cache is None:
        _nc_cache = _build_nc()
    return _nc_cache


def _softmax(w):
    m = w.max(axis=1, keepdims=True)
    e = np.exp(w - m)
    return e / e.sum(axis=1, keepdims=True)


def _run(A, X, conv_w_l0_1, conv_w_l0_2, conv_w_l1, gcn_weight, trace=False):
    _install_ntff_hook()
    from concourse.bass_utils import run_bass_kernel_spmd

    bf16 = ml_dtypes.bfloat16
    A = np.ascontiguousarray(np.asarray(A, np.float32)).astype(bf16)
    X = np.asarray(X, np.float32)
    s1 = _softmax(np.asarray(conv_w_l0_1, np.float32)[:, :, 0, 0])  # [2, 5]
    s2 = _softmax(np.asarray(conv_w_l0_2, np.float32)[:, :, 0, 0])
    s3 = _softmax(np.asarray(conv_w_l1, np.float32)[:, :, 0, 0])
    w = np.ascontiguousarray(np.asarray(gcn_weight, np.float32)).astype(bf16)

    in_maps = []
    for c in range(8):
        r, g = c % 4, c // 4
        rows = slice(S * r, S * (r + 1))
        in_maps.append({
            "a_rows": np.ascontiguousarray(A[:, rows, :]),
            "s1": np.ascontiguousarray(np.broadcast_to(s1[g], (P, E))).astype(np.float32),
            "s2": np.ascontiguousarray(np.broadcast_to(s2[g], (P, E))).astype(np.float32),
            "s3": np.ascontiguousarray(np.broadcast_to(s3[g], (P, E))).astype(np.float32),
            "xt": np.ascontiguousarray(X[rows, :].T.astype(bf16)),
            "w": w,
            "doff": np.ascontiguousarray(np.broadcast_to(
                (S * r + P * np.arange(4, dtype=np.float32))[None, :],
                (P, 4))).astype(np.float32),
        })

    nc = _get_nc()
    res = run_bass_kernel_spmd(nc, in_maps, core_ids=list(range(8)), trace=trace)
    out = np.concatenate([res.results[0]["out"], res.results[4]["out"]], axis=1)
    return np.ascontiguousarray(out.astype(np.float32)), res


def kernel(A, X, conv_w_l0_1, conv_w_l0_2, conv_w_l1, gcn_weight):
    out, _ = _run(A, X, conv_w_l0_1, conv_w_l0_2, conv_w_l1, gcn_weight)
    return out



# revision 2
# speedup vs baseline: 1.4437x; 1.4437x over previous
"""GTN (Graph Transformer Network) kernel on 8 TRN2 NeuronCores via Bass/Tile.

Problem nn_GTN_17162689314910:
  A: [E=5, N=2048, N] f32, X: [N, 256] f32, conv_w_*: [C=2, E, 1, 1] f32,
  gcn_weight: [256, 64] f32 -> out [N, C*64] f32.

Math (per channel c):
  a = sum_e softmax(w1)[c,e] A[e];  b, a1 likewise with w2, w3
  H0 = a @ b
  H0n = H0 * 1/(colsum(H0)+eps)          (norm add=False; diag term dropped)
  H1 = H0n @ a1
  H1d = H1 with diag set to 1
  out_c = relu(H1d^T @ (X @ W) * 1/(colsum(H1d)+eps)[:,None])

Sharding: channel-split. Cores 0-3 = channel 0, cores 4-7 = channel 1;
within a group, 512-row shards. bf16 compute, fp32 PSUM.

v2 schedule (single A pass, consolidated collectives, H0T formulation):
  - One streaming pass over A computes all three convs: conv_b on PE
    (5 accumulating matmuls with diag(s_e) stationary), conv_a split
    DVE/PE, conv_a1 on DVE.
  - Two 8-core AllGathers (b then a1), 2MB/rank each, Shared outputs.
  - mm1 computes H0T = (a@b)^T directly: lhsT = gathered b panels
    (natural layout), rhs = aT. No second transpose round: H0T is
    exactly mm2's lhsT, and mm2's output H1 is natural for readout.
  - deg0 = colsum(H0) = free-dim rowsum of H0T (DVE) + 8KB group-4
    AllReduce; reciprocal on [128,16] layout (not [1,2048]).
  - readout partials ReduceScatter'd (group-4); each core emits only
    its own 512-row strip of the output.
  - PSUM->SBUF copies ride the Scalar engine; DMA is split across the
    two HWDGE rings (sync: A/b-cache/a1-panels, scalar: small/comm).
"""
import sys
import types

import numpy as np
import ml_dtypes

P = 128
N = 2048
S = 512            # shard rows per core
E = 5
TK = N // P        # 16 k tiles
TI = S // P        # 4 i tiles
Q = 512            # mm column-quarter width
EPS = 1e-8
GROUPS8 = [[0, 1, 2, 3, 4, 5, 6, 7]]
GROUPS4 = [[0, 1, 2, 3], [4, 5, 6, 7]]

_nc_cache = None


def _install_ntff_hook():
    if "antenv.axon_hooks" in sys.modules:
        return
    try:
        from trn_agent_boot.trn_boot import _ntff_profile_via_ctypes
        hook = _ntff_profile_via_ctypes("/opt/axon/libaxon_pjrt.so")
    except Exception:
        hook = None
    mod = types.ModuleType("antenv.axon_hooks")
    mod.get_axon_ntff_profile_hook = lambda: hook
    mod.set_axon_ntff_profile_hook = lambda h: None
    sys.modules["antenv.axon_hooks"] = mod


def _build_nc():
    import concourse.mybir as mybir
    import concourse.tile as tile
    from concourse import bacc
    from concourse.bass import ds
    from concourse.masks import make_identity

    bf16 = mybir.dt.bfloat16
    f32 = mybir.dt.float32
    u8 = mybir.dt.uint8
    Alu = mybir.AluOpType
    Act = mybir.ActivationFunctionType

    nc = bacc.Bacc(None)
    nc.num_devices = 8

    a_rows = nc.dram_tensor("a_rows", [E, S, N], bf16, kind="ExternalInput")
    s1 = nc.dram_tensor("s1", [P, E], f32, kind="ExternalInput")
    s2 = nc.dram_tensor("s2", [P, E], f32, kind="ExternalInput")
    s3 = nc.dram_tensor("s3", [P, E], f32, kind="ExternalInput")
    xt = nc.dram_tensor("xt", [256, S], bf16, kind="ExternalInput")
    w_in = nc.dram_tensor("w", [256, 64], bf16, kind="ExternalInput")
    doff = nc.dram_tensor("doff", [P, TI], f32, kind="ExternalInput")
    out = nc.dram_tensor("out", [S, 64], f32, kind="ExternalOutput")

    with tile.TileContext(nc) as tc:
        with (
            tc.tile_pool(name="pers", bufs=1) as pers,
            tc.tile_pool(name="work", bufs=2) as work,
            tc.tile_pool(name="big", bufs=4) as bigp,
            tc.tile_pool(name="pan", bufs=4) as panp,
            tc.tile_pool(name="ps", bufs=8, space="PSUM") as psp,
            tc.tile_pool(name="dram", bufs=1, space="DRAM") as dram,
        ):
            pid = nc.partition_id()
            g4 = (pid // 4) * (4 * S)     # row base of own group in AG outputs

            # ---- small constants (scalar HWDGE ring) ----
            s1_sb = pers.tile([P, E], f32, name="s1_sb")
            s2_sb = pers.tile([P, E], f32, name="s2_sb")
            s3_sb = pers.tile([P, E], f32, name="s3_sb")
            doff_sb = pers.tile([P, TI], f32, name="doff_sb")
            nc.scalar.dma_start(s1_sb[:], s1[:])
            nc.scalar.dma_start(s2_sb[:], s2[:])
            nc.scalar.dma_start(s3_sb[:], s3[:])
            nc.scalar.dma_start(doff_sb[:], doff[:])
            xt_sb = [pers.tile([P, S], bf16, name=f"xt_{k}") for k in range(2)]
            w_sb = [pers.tile([P, 64], bf16, name=f"w_{k}") for k in range(2)]
            for k in range(2):
                nc.scalar.dma_start(xt_sb[k][:], xt[P * k:P * (k + 1), :])
                nc.scalar.dma_start(w_sb[k][:], w_in[P * k:P * (k + 1), :])

            ident = pers.tile([P, P], bf16, name="ident")
            make_identity(nc, ident)
            ones_q = pers.tile([P, Q], bf16, name="ones_q")
            nc.gpsimd.memset(ones_q[:], 1.0)

            # diag(s_j[e]) stationary tiles for PE conv
            dscb = [pers.tile([P, P], bf16, name=f"dscb_{e}") for e in range(E)]
            dsca = [pers.tile([P, P], bf16, name=f"dsca_{e}") for e in range(E)]
            for e in range(E):
                nc.vector.tensor_scalar(
                    dscb[e][:], ident[:], s2_sb[:, e:e + 1], None, op0=Alu.mult)
                nc.vector.tensor_scalar(
                    dsca[e][:], ident[:], s1_sb[:, e:e + 1], None, op0=Alu.mult)

            # diag-position masks: mask[t][p, c] = (c - p == doff[t])
            masks = [pers.tile([P, N], u8, name=f"mask_{t}") for t in range(TI)]
            with tc.tile_pool(name="iotap", bufs=1) as iotap:
                iota_f = iotap.tile([P, N], f32, name="iota_f")
                nc.gpsimd.iota(iota_f[:], pattern=[[1, N]], base=0,
                               channel_multiplier=-1,
                               allow_small_or_imprecise_dtypes=True)
                for t in range(TI):
                    nc.vector.tensor_scalar(
                        masks[t][:], iota_f[:], doff_sb[:, t:t + 1], None,
                        op0=Alu.is_equal)

            # ---- collective DRAM buffers ----
            agb_in = dram.tile([S, N], bf16, name="agb_in")
            agb_out = dram.tile([8 * S, N], bf16, name="agb_out",
                                addr_space="Shared")
            aga_in = dram.tile([S, N], bf16, name="aga_in")
            aga_out = dram.tile([8 * S, N], bf16, name="aga_out",
                                addr_space="Shared")
            deg_in = dram.tile([P, TK], f32, name="deg_in")
            deg_out = dram.tile([P, TK], f32, name="deg_out")
            c2_in = dram.tile([N, 65], f32, name="c2_in")
            c2_out = dram.tile([S, 65], f32, name="c2_out")

            # ---- A tiles: 4 row-tiles x 5 channels, single load ----
            At = [bigp.tile([P, E * N], bf16, name="At", tag="big")
                  for _ in range(TI)]
            for t in range(TI):
                for e in range(E):
                    nc.sync.dma_start(At[t][:, e * N:(e + 1) * N],
                                      a_rows[e, P * t:P * (t + 1), :])

            def conv_pe(dst_sb, t, dscs):
                # dst = sum_e s_e * A[e] via accumulating matmuls,
                # lhsT = diag(s_e) stationary, rhs = A tile quarters.
                for q in range(4):
                    cv = psp.tile([P, Q], f32, name="cv", tag="ps")
                    for e in range(E):
                        nc.tensor.matmul(
                            cv[:], dscs[e][:],
                            At[t][:, e * N + Q * q:e * N + Q * (q + 1)],
                            start=(e == 0), stop=(e == E - 1))
                    nc.scalar.copy(dst_sb[:, Q * q:Q * (q + 1)], cv[:])

            def conv_dve(dst_sb, t, s_ap):
                nc.vector.tensor_scalar(
                    dst_sb[:], At[t][:, 0:N], s_ap[:, 0:1], None, op0=Alu.mult)
                for e in range(1, E):
                    nc.vector.scalar_tensor_tensor(
                        dst_sb[:], At[t][:, e * N:(e + 1) * N], s_ap[:, e:e + 1],
                        dst_sb[:], op0=Alu.mult, op1=Alu.add)

            # ---- conv_b on PE -> AG_B ASAP ----
            for t in range(TI):
                bt = work.tile([P, N], bf16, name="bt", tag="bt")
                conv_pe(bt, t, dscb)
                nc.scalar.dma_start(agb_in[P * t:P * (t + 1), :], bt[:])
            nc.gpsimd.collective_compute(
                "AllGather", Alu.bypass, replica_groups=GROUPS8,
                ins=[agb_in.opt()], outs=[agb_out.opt()])

            # ---- conv_a: DVE t0,t1 + PE t2,t3 ----
            a_sb = [pers.tile([P, N], bf16, name=f"a_sb{t}") for t in range(TI)]
            conv_dve(a_sb[0], 0, s1_sb)
            conv_dve(a_sb[1], 1, s1_sb)
            conv_pe(a_sb[2], 2, dsca)
            conv_pe(a_sb[3], 3, dsca)

            # ---- conv_a1 on DVE -> AG_A1 ----
            for t in range(TI):
                a1t = work.tile([P, N], bf16, name="a1t", tag="a1t")
                conv_dve(a1t, t, s3_sb)
                nc.scalar.dma_start(aga_in[P * t:P * (t + 1), :], a1t[:])
            nc.gpsimd.collective_compute(
                "AllGather", Alu.bypass, replica_groups=GROUPS8,
                ins=[aga_in.opt()], outs=[aga_out.opt()])

            # ---- aT via PE transposes ----
            aT = [pers.tile([P, S], bf16, name=f"aT_{k}") for k in range(TK)]
            for t in range(TI):
                for k in range(TK):
                    pt = psp.tile([P, P], bf16, name="pt", tag="ps")
                    nc.tensor.transpose(pt[:], a_sb[t][:, P * k:P * (k + 1)],
                                        ident[:])
                    nc.scalar.copy(aT[k][:, P * t:P * (t + 1)], pt[:])

            # ---- Xw = (X @ W)[own rows], col 64 = 1 (colsum rider) ----
            xwo = [pers.tile([P, 65], bf16, name=f"xwo_{t}") for t in range(TI)]
            for t in range(TI):
                nc.gpsimd.memset(xwo[t][:, 64:65], 1.0)
            for t in range(TI):
                px = psp.tile([P, 64], f32, name="px", tag="ps")
                for k in range(2):
                    nc.tensor.matmul(px[:], xt_sb[k][:, P * t:P * (t + 1)],
                                     w_sb[k][:], start=(k == 0), stop=(k == 1))
                nc.scalar.copy(xwo[t][:, 0:64], px[:])

            # ---- b cache from AG_B output (reuses A-pool slots) ----
            bcg = [bigp.tile([P, 4 * N], bf16, name="At", tag="big")
                   for _ in range(4)]
            for kb in range(4):
                for j in range(4):
                    k = 4 * kb + j
                    nc.sync.dma_start(bcg[kb][:, j * N:(j + 1) * N],
                                      agb_out[ds(g4 + P * k, P), :])

            def bc_sl(k, ib):
                return bcg[k // 4][:, (k % 4) * N + P * ib:(k % 4) * N + P * (ib + 1)]

            # ---- mm1: H0T[ib] = sum_k b[kblk, ib]^T-contracted with aT[k] ----
            #      out tile ib holds H0T[128ib:128(ib+1), own 512 rows]
            h0T = [pers.tile([P, S], bf16, name=f"h0T_{k}") for k in range(TK)]
            deg_sbT = pers.tile([P, TK], f32, name="deg_sbT")
            for chunk in (range(0, 6), range(6, 12), range(12, 16)):
                h0ps = {ib: psp.tile([P, S], f32, name=f"h0ps{ib}", tag="ps")
                        for ib in chunk}
                for k in range(TK):
                    for ib in chunk:
                        nc.tensor.matmul(h0ps[ib][:], bc_sl(k, ib), aT[k][:],
                                         start=(k == 0), stop=(k == TK - 1))
                for ib in chunk:
                    nc.scalar.copy(h0T[ib][:], h0ps[ib][:])
                    nc.vector.tensor_reduce(
                        deg_sbT[:, ib:ib + 1], h0T[ib][:],
                        mybir.AxisListType.X, Alu.add)

            # ---- deg0 AllReduce (8KB, group-4) -> dinv0, scale H0T ----
            nc.scalar.dma_start(deg_in[:], deg_sbT[:])
            nc.gpsimd.collective_compute(
                "AllReduce", Alu.add, replica_groups=GROUPS4,
                ins=[deg_in.opt()], outs=[deg_out.opt()])
            dinvT = pers.tile([P, TK], f32, name="dinvT")
            nc.scalar.dma_start(dinvT[:], deg_out[:])
            nc.vector.tensor_scalar(dinvT[:], dinvT[:], float(EPS), None,
                                    op0=Alu.add)
            nc.vector.reciprocal(dinvT[:], dinvT[:])
            for ib in range(TK):
                nc.vector.tensor_scalar(h0T[ib][:], h0T[ib][:],
                                        dinvT[:, ib:ib + 1], None, op0=Alu.mult)

            # ---- mm2 + diag fix + readout, pipelined per column-quarter ----
            H1 = [pers.tile([P, N], bf16, name=f"H1_{t}") for t in range(TI)]
            for q in range(4):
                pts = [psp.tile([P, Q], f32, name=f"pt2_{i}", tag="ps")
                       for i in range(TI)]
                for k in range(TK):
                    pan = panp.tile([P, Q], bf16, name="pan", tag="pan")
                    nc.sync.dma_start(pan[:],
                                      aga_out[ds(g4 + P * k, P), Q * q:Q * (q + 1)])
                    for i in range(TI):
                        nc.tensor.matmul(pts[i][:], h0T[k][:, P * i:P * (i + 1)],
                                         pan[:], start=(k == 0), stop=(k == TK - 1))
                for i in range(TI):
                    nc.scalar.copy(H1[i][:, Q * q:Q * (q + 1)], pts[i][:])
                    nc.vector.copy_predicated(H1[i][:, Q * q:Q * (q + 1)],
                                              masks[i][:, Q * q:Q * (q + 1)],
                                              ones_q[:])
                for jb in range(4 * q, 4 * q + 4):
                    pr = psp.tile([P, 65], f32, name="pr", tag="ps")
                    for i in range(TI):
                        nc.tensor.matmul(pr[:], H1[i][:, P * jb:P * (jb + 1)],
                                         xwo[i][:], start=(i == 0), stop=(i == TI - 1))
                    ro = work.tile([P, 65], f32, name="ro", tag="ro")
                    nc.scalar.copy(ro[:], pr[:])
                    nc.scalar.dma_start(c2_in[P * jb:P * (jb + 1), :], ro[:])

            # ---- ReduceScatter readout partials (group-4) ----
            nc.gpsimd.collective_compute(
                "ReduceScatter", Alu.add, replica_groups=GROUPS4,
                ins=[c2_in.opt()], outs=[c2_out.opt()])

            # ---- final: relu(partial * deginv1), own 512 rows only ----
            fo = pers.tile([P, TI * 65], f32, name="fo")
            for j in range(TI):
                nc.scalar.dma_start(fo[:, j * 65:(j + 1) * 65],
                                    c2_out[P * j:P * (j + 1), :])
            dinv1 = pers.tile([P, TI], f32, name="dinv1")
            nc.vector.tensor_scalar(
                dinv1[:], fo[:, 64::65], float(EPS), None, op0=Alu.add)
            nc.vector.reciprocal(dinv1[:], dinv1[:])
            for j in range(TI):
                oj = work.tile([P, 64], f32, name="oj", tag="oj")
                nc.scalar.activation(oj[:], fo[:, j * 65:j * 65 + 64],
                                     Act.Relu, scale=dinv1[:, j:j + 1])
                nc.scalar.dma_start(out[P * j:P * (j + 1), :], oj[:])

    nc.finalize()
    return nc


def _get_nc():
    global _nc_cache
    if _nc_cache is None:
        _nc_cache = _build_nc()
    return _nc_cache


def _softmax(w):
    m = w.max(axis=1, keepdims=True)
    e = np.exp(w - m)
    return e / e.sum(axis=1, keepdims=True)


def _run(A, X, conv_w_l0_1, conv_w_l0_2, conv_w_l1, gcn_weight, trace=False):
    _install_ntff_hook()
    from concourse.bass_utils import run_bass_kernel_spmd

    bf16 = ml_dtypes.bfloat16
    A = np.ascontiguousarray(np.asarray(A, np.float32)).astype(bf16)
    X = np.asarray(X, np.float32)
    s1 = _softmax(np.asarray(conv_w_l0_1, np.float32)[:, :, 0, 0])  # [2, 5]
    s2 = _softmax(np.asarray(conv_w_l0_2, np.float32)[:, :, 0, 0])
    s3 = _softmax(np.asarray(conv_w_l1, np.float32)[:, :, 0, 0])
    w = np.ascontiguousarray(np.asarray(gcn_weight, np.float32)).astype(bf16)

    in_maps = []
    for c in range(8):
        r, g = c % 4, c // 4
        rows = slice(S * r, S * (r + 1))
        in_maps.append({
            "a_rows": np.ascontiguousarray(A[:, rows, :]),
            "s1": np.ascontiguousarray(np.broadcast_to(s1[g], (P, E))).astype(np.float32),
            "s2": np.ascontiguousarray(np.broadcast_to(s2[g], (P, E))).astype(np.float32),
            "s3": np.ascontiguousarray(np.broadcast_to(s3[g], (P, E))).astype(np.float32),
            "xt": np.ascontiguousarray(X[rows, :].T.astype(bf16)),
            "w": w,
            "doff": np.ascontiguousarray(np.broadcast_to(
                (S * r + P * np.arange(4, dtype=np.float32))[None, :],
                (P, 4))).astype(np.float32),
        })

    nc = _get_nc()
    res = run_bass_kernel_spmd(nc, in_maps, core_ids=list(range(8)), trace=trace)
    full = np.empty((N, 128), np.float32)
    for c in range(8):
        r, g = c % 4, c // 4
        full[S * r:S * (r + 1), 64 * g:64 * (g + 1)] = res.results[c]["out"]
    return np.ascontiguousarray(full), res


def kernel(A, X, conv_w_l0_1, conv_w_l0_2, conv_w_l1, gcn_weight):
    out, _ = _run(A, X, conv_w_l0_1, conv_w_l0_2, conv_w_l1, gcn_weight)
    return out


# revision 5
# speedup vs baseline: 1.4737x; 1.0208x over previous
"""GTN (Graph Transformer Network) kernel on 8 TRN2 NeuronCores via Bass/Tile.

Problem nn_GTN_17162689314910:
  A: [E=5, N=2048, N] f32, X: [N, 256] f32, conv_w_*: [C=2, E, 1, 1] f32,
  gcn_weight: [256, 64] f32 -> out [N, C*64] f32.

Math (per channel c):
  a = sum_e softmax(w1)[c,e] A[e];  b, a1 likewise with w2, w3
  H0 = a @ b
  H0n = H0 * 1/(colsum(H0)+eps)          (norm add=False; diag term dropped)
  H1 = H0n @ a1
  H1d = H1 with diag set to 1
  out_c = relu(H1d^T @ (X @ W) * 1/(colsum(H1d)+eps)[:,None])

Sharding: channel-split. Cores 0-3 = channel 0, cores 4-7 = channel 1;
within a group, 512-row shards. bf16 compute, fp32 PSUM.

v2 schedule (single A pass, consolidated collectives, H0T formulation):
  - One streaming pass over A computes all three convs: conv_b on PE
    (5 accumulating matmuls with diag(s_e) stationary), conv_a split
    DVE/PE, conv_a1 on DVE.
  - Two 8-core AllGathers (b then a1), 2MB/rank each, Shared outputs.
  - mm1 computes H0T = (a@b)^T directly: lhsT = gathered b panels
    (natural layout), rhs = aT. No second transpose round: H0T is
    exactly mm2's lhsT, and mm2's output H1 is natural for readout.
  - deg0 = colsum(H0) = free-dim rowsum of H0T (DVE) + 8KB group-4
    AllReduce; reciprocal on [128,16] layout (not [1,2048]).
  - readout partials ReduceScatter'd (group-4); each core emits only
    its own 512-row strip of the output.
  - PSUM->SBUF copies ride the Scalar engine; DMA is split across the
    two HWDGE rings (sync: A/b-cache/a1-panels, scalar: small/comm).
"""
import sys
import types

import numpy as np
import ml_dtypes

P = 128
N = 2048
S = 512            # shard rows per core
E = 5
TK = N // P        # 16 k tiles
TI = S // P        # 4 i tiles
Q = 512            # mm column-quarter width
EPS = 1e-8
GROUPS8 = [[0, 1, 2, 3, 4, 5, 6, 7]]
GROUPS4 = [[0, 1, 2, 3], [4, 5, 6, 7]]

_nc_cache = None


def _install_ntff_hook():
    if "antenv.axon_hooks" in sys.modules:
        return
    try:
        from trn_agent_boot.trn_boot import _ntff_profile_via_ctypes
        hook = _ntff_profile_via_ctypes("/opt/axon/libaxon_pjrt.so")
    except Exception:
        hook = None
    mod = types.ModuleType("antenv.axon_hooks")
    mod.get_axon_ntff_profile_hook = lambda: hook
    mod.set_axon_ntff_profile_hook = lambda h: None
    sys.modules["antenv.axon_hooks"] = mod


def _build_nc():
    import concourse.mybir as mybir
    import concourse.tile as tile
    from concourse import bacc
    from concourse.bass import ds
    from concourse.masks import make_identity

    bf16 = mybir.dt.bfloat16
    f32 = mybir.dt.float32
    u8 = mybir.dt.uint8
    Alu = mybir.AluOpType
    Act = mybir.ActivationFunctionType

    nc = bacc.Bacc(None)
    nc.num_devices = 8

    a_rows = nc.dram_tensor("a_rows", [E, S, N], bf16, kind="ExternalInput")
    s1 = nc.dram_tensor("s1", [P, E], f32, kind="ExternalInput")
    s2 = nc.dram_tensor("s2", [P, E], f32, kind="ExternalInput")
    s3 = nc.dram_tensor("s3", [P, E], f32, kind="ExternalInput")
    xt = nc.dram_tensor("xt", [256, S], bf16, kind="ExternalInput")
    w_in = nc.dram_tensor("w", [256, 64], bf16, kind="ExternalInput")
    doff = nc.dram_tensor("doff", [P, TI], f32, kind="ExternalInput")
    out = nc.dram_tensor("out", [S, 64], f32, kind="ExternalOutput")

    with tile.TileContext(nc) as tc:
        with (
            tc.tile_pool(name="pers", bufs=1) as pers,
            tc.tile_pool(name="work", bufs=2) as work,
            tc.tile_pool(name="big", bufs=4) as bigp,
            tc.tile_pool(name="pan", bufs=4) as panp,
            tc.tile_pool(name="ps", bufs=8, space="PSUM") as psp,
            tc.tile_pool(name="dram", bufs=1, space="DRAM") as dram,
        ):
            pid = nc.partition_id()
            g4 = (pid // 4) * (4 * S)     # row base of own group in AG outputs

            # ---- small constants (scalar HWDGE ring) ----
            s1_sb = pers.tile([P, E], f32, name="s1_sb")
            s2_sb = pers.tile([P, E], f32, name="s2_sb")
            s3_sb = pers.tile([P, E], f32, name="s3_sb")
            doff_sb = pers.tile([P, TI], f32, name="doff_sb")
            nc.scalar.dma_start(s1_sb[:], s1[:])
            nc.scalar.dma_start(s2_sb[:], s2[:])
            nc.scalar.dma_start(s3_sb[:], s3[:])
            nc.scalar.dma_start(doff_sb[:], doff[:])
            xt_sb = [pers.tile([P, S], bf16, name=f"xt_{k}") for k in range(2)]
            w_sb = [pers.tile([P, 64], bf16, name=f"w_{k}") for k in range(2)]
            for k in range(2):
                nc.scalar.dma_start(xt_sb[k][:], xt[P * k:P * (k + 1), :])
                nc.scalar.dma_start(w_sb[k][:], w_in[P * k:P * (k + 1), :])

            ident = pers.tile([P, P], bf16, name="ident")
            make_identity(nc, ident)

            # diag(s_j[e]) stationary tiles for PE conv
            dscb = [pers.tile([P, P], bf16, name=f"dscb_{e}") for e in range(E)]
            dsca = [pers.tile([P, P], bf16, name=f"dsca_{e}") for e in range(E)]
            for e in range(E):
                nc.vector.tensor_scalar(
                    dscb[e][:], ident[:], s2_sb[:, e:e + 1], None, op0=Alu.mult)
                nc.vector.tensor_scalar(
                    dsca[e][:], ident[:], s1_sb[:, e:e + 1], None, op0=Alu.mult)

            # ---- collective DRAM buffers ----
            # b gathered in two row-chunks so the first collective can
            # launch after only 2 conv tiles (and mm1 can start on half
            # the k-blocks while chunk 2 is in flight).
            agb1_in = dram.tile([2 * P, N], bf16, name="agb1_in")
            agb2_in = dram.tile([2 * P, N], bf16, name="agb2_in")
            agb1_out = dram.tile([16 * P, N], bf16, name="agb1_out",
                                 addr_space="Shared")
            agb2_out = dram.tile([16 * P, N], bf16, name="agb2_out",
                                 addr_space="Shared")
            aga_in = dram.tile([S, N], bf16, name="aga_in")
            aga_out = dram.tile([8 * S, N], bf16, name="aga_out",
                                addr_space="Shared")
            deg_in1 = dram.tile([P, 8], f32, name="deg_in1")
            deg_out1 = dram.tile([P, 8], f32, name="deg_out1")
            deg_in2 = dram.tile([P, 8], f32, name="deg_in2")
            deg_out2 = dram.tile([P, 8], f32, name="deg_out2")
            c2_in = dram.tile([N, 65], f32, name="c2_in")
            c2_out = dram.tile([S, 65], f32, name="c2_out")

            # ---- A tiles: 4 row-tiles x 5 channels, single load ----
            At = [bigp.tile([P, E * N], bf16, name="At", tag="big")
                  for _ in range(TI)]
            for t in range(TI):
                for e in range(E):
                    nc.sync.dma_start(At[t][:, e * N:(e + 1) * N],
                                      a_rows[e, P * t:P * (t + 1), :])

            def conv_pe(dst_sb, t, dscs):
                # dst = sum_e s_e * A[e] via accumulating matmuls,
                # lhsT = diag(s_e) stationary, rhs = A tile quarters.
                for q in range(4):
                    cv = psp.tile([P, Q], f32, name="cv", tag="ps")
                    for e in range(E):
                        nc.tensor.matmul(
                            cv[:], dscs[e][:],
                            At[t][:, e * N + Q * q:e * N + Q * (q + 1)],
                            start=(e == 0), stop=(e == E - 1))
                    nc.scalar.copy(dst_sb[:, Q * q:Q * (q + 1)], cv[:])

            def conv_dve(dst_sb, t, s_ap):
                nc.vector.tensor_scalar(
                    dst_sb[:], At[t][:, 0:N], s_ap[:, 0:1], None, op0=Alu.mult)
                for e in range(1, E):
                    nc.vector.scalar_tensor_tensor(
                        dst_sb[:], At[t][:, e * N:(e + 1) * N], s_ap[:, e:e + 1],
                        dst_sb[:], op0=Alu.mult, op1=Alu.add)

            # ---- conv_b on PE -> chunked AG_B ASAP ----
            for t in range(TI):
                bt = work.tile([P, N], bf16, name="bt", tag="bt")
                conv_pe(bt, t, dscb)
                dst = agb1_in if t < 2 else agb2_in
                nc.scalar.dma_start(dst[P * (t % 2):P * (t % 2 + 1), :], bt[:])
                if t == 1:
                    nc.gpsimd.collective_compute(
                        "AllGather", Alu.bypass, replica_groups=GROUPS8,
                        ins=[agb1_in.opt()], outs=[agb1_out.opt()])
                elif t == 3:
                    nc.gpsimd.collective_compute(
                        "AllGather", Alu.bypass, replica_groups=GROUPS8,
                        ins=[agb2_in.opt()], outs=[agb2_out.opt()])

            # ---- conv_a: DVE t0,t1 + PE t2,t3 ----
            a_sb = [pers.tile([P, N], bf16, name=f"a_sb{t}") for t in range(TI)]
            conv_dve(a_sb[0], 0, s1_sb)
            conv_dve(a_sb[1], 1, s1_sb)
            conv_pe(a_sb[2], 2, dsca)
            conv_pe(a_sb[3], 3, dsca)

            # ---- conv_a1 on DVE -> AG_A1 ----
            for t in range(TI):
                a1t = work.tile([P, N], bf16, name="a1t", tag="a1t")
                conv_dve(a1t, t, s3_sb)
                nc.scalar.dma_start(aga_in[P * t:P * (t + 1), :], a1t[:])
            nc.gpsimd.collective_compute(
                "AllGather", Alu.bypass, replica_groups=GROUPS8,
                ins=[aga_in.opt()], outs=[aga_out.opt()])

            # diag-position masks: mask[t][p, c] = (c - p == doff[t]).
            # gpsimd work sits after the AG triggers so it can't delay them.
            ones_q = pers.tile([P, Q], bf16, name="ones_q")
            nc.gpsimd.memset(ones_q[:], 1.0)
            masks = [pers.tile([P, N], u8, name=f"mask_{t}") for t in range(TI)]
            with tc.tile_pool(name="iotap", bufs=1) as iotap:
                iota_f = iotap.tile([P, N], f32, name="iota_f")
                nc.gpsimd.iota(iota_f[:], pattern=[[1, N]], base=0,
                               channel_multiplier=-1,
                               allow_small_or_imprecise_dtypes=True)
                for t in range(TI):
                    nc.vector.tensor_scalar(
                        masks[t][:], iota_f[:], doff_sb[:, t:t + 1], None,
                        op0=Alu.is_equal)

            # ---- aT via PE transposes ----
            aT = [pers.tile([P, S], bf16, name=f"aT_{k}") for k in range(TK)]
            for t in range(TI):
                for k in range(TK):
                    pt = psp.tile([P, P], bf16, name="pt", tag="ps")
                    nc.tensor.transpose(pt[:], a_sb[t][:, P * k:P * (k + 1)],
                                        ident[:])
                    nc.scalar.copy(aT[k][:, P * t:P * (t + 1)], pt[:])

            # ---- Xw = (X @ W)[own rows], col 64 = 1 (colsum rider) ----
            xwo = [pers.tile([P, 65], bf16, name=f"xwo_{t}") for t in range(TI)]
            for t in range(TI):
                nc.gpsimd.memset(xwo[t][:, 64:65], 1.0)
            for t in range(TI):
                px = psp.tile([P, 64], f32, name="px", tag="ps")
                for k in range(2):
                    nc.tensor.matmul(px[:], xt_sb[k][:, P * t:P * (t + 1)],
                                     w_sb[k][:], start=(k == 0), stop=(k == 1))
                nc.scalar.copy(xwo[t][:, 0:64], px[:])

            # ---- b cache from AG_B outputs (reuses A-pool slots) ----
            # slot h: h0 = chunk1 ranks 0,1 (k 0,1,4,5), h1 = chunk1
            # ranks 2,3 (k 8,9,12,13), h2/h3 likewise from chunk2.
            g4b = (pid // 4) * (8 * P)    # group row base in chunk outputs
            bcg = [bigp.tile([P, 4 * N], bf16, name="At", tag="big")
                   for _ in range(4)]
            for h in range(4):
                src = agb1_out if h < 2 else agb2_out
                for idx in range(4):
                    q = 2 * (h % 2) + idx // 2
                    jj = idx % 2
                    nc.sync.dma_start(
                        bcg[h][:, idx * N:(idx + 1) * N],
                        src[ds(g4b + 2 * P * q + P * jj, P), :])

            def bc_sl(k, ib):
                q, j = k // 4, k % 4
                h = 2 * (j // 2) + q // 2
                idx = 2 * (q % 2) + j % 2
                return bcg[h][:, idx * N + P * ib:idx * N + P * (ib + 1)]

            KORDER = [4 * q + j for jh in (0, 2) for qh in (0, 2)
                      for q in (qh, qh + 1) for j in (jh, jh + 1)]

            # ---- mm1: H0T[ib] = sum_k b[kblk, ib]^T-contracted with aT[k] ----
            #      out tile ib holds H0T[128ib:128(ib+1), own 512 rows]
            h0T = [pers.tile([P, S], bf16, name=f"h0T_{k}") for k in range(TK)]
            deg_sbT = pers.tile([P, TK], f32, name="deg_sbT")
            dinvT = pers.tile([P, TK], f32, name="dinvT")
            for ci, chunk in enumerate((range(0, 8), range(8, 16))):
                h0ps = {ib: psp.tile([P, S], f32, name=f"h0ps{ib}", tag="ps")
                        for ib in chunk}
                for ki, k in enumerate(KORDER):
                    for ib in chunk:
                        nc.tensor.matmul(h0ps[ib][:], bc_sl(k, ib), aT[k][:],
                                         start=(ki == 0), stop=(ki == TK - 1))
                for ib in chunk:
                    nc.scalar.copy(h0T[ib][:], h0ps[ib][:])
                    nc.vector.tensor_reduce(
                        deg_sbT[:, ib:ib + 1], h0T[ib][:],
                        mybir.AxisListType.X, Alu.add)
                # ---- deg0 AllReduce half (4KB, group-4) -> dinv0 half,
                #      scale the H0T half (AR of half 1 hides under half-2
                #      compute; mm2 k 0-7 can start before AR of half 2) ----
                dgi = deg_in1 if ci == 0 else deg_in2
                dgo = deg_out1 if ci == 0 else deg_out2
                lo = 8 * ci
                nc.scalar.dma_start(dgi[:], deg_sbT[:, lo:lo + 8])
                nc.gpsimd.collective_compute(
                    "AllReduce", Alu.add, replica_groups=GROUPS4,
                    ins=[dgi.opt()], outs=[dgo.opt()])
                nc.scalar.dma_start(dinvT[:, lo:lo + 8], dgo[:])
                nc.vector.tensor_scalar(dinvT[:, lo:lo + 8], dinvT[:, lo:lo + 8],
                                        float(EPS), None, op0=Alu.add)
                nc.vector.reciprocal(dinvT[:, lo:lo + 8], dinvT[:, lo:lo + 8])
                for ib in chunk:
                    nc.vector.tensor_scalar(h0T[ib][:], h0T[ib][:],
                                            dinvT[:, ib:ib + 1], None,
                                            op0=Alu.mult)

            # ---- mm2 + diag fix + readout, pipelined per column-quarter ----
            H1 = [pers.tile([P, N], bf16, name=f"H1_{t}") for t in range(TI)]
            for q in range(4):
                pts = [psp.tile([P, Q], f32, name=f"pt2_{i}", tag="ps")
                       for i in range(TI)]
                for k in range(TK):
                    pan = panp.tile([P, Q], bf16, name="pan", tag="pan")
                    nc.sync.dma_start(pan[:],
                                      aga_out[ds(g4 + P * k, P), Q * q:Q * (q + 1)])
                    for i in range(TI):
                        nc.tensor.matmul(pts[i][:], h0T[k][:, P * i:P * (i + 1)],
                                         pan[:], start=(k == 0), stop=(k == TK - 1))
                for i in range(TI):
                    nc.scalar.copy(H1[i][:, Q * q:Q * (q + 1)], pts[i][:])
                    nc.vector.copy_predicated(H1[i][:, Q * q:Q * (q + 1)],
                                              masks[i][:, Q * q:Q * (q + 1)],
                                              ones_q[:])
                for jb in range(4 * q, 4 * q + 4):
                    pr = psp.tile([P, 65], f32, name="pr", tag="ps")
                    for i in range(TI):
                        nc.tensor.matmul(pr[:], H1[i][:, P * jb:P * (jb + 1)],
                                         xwo[i][:], start=(i == 0), stop=(i == TI - 1))
                    ro = work.tile([P, 65], f32, name="ro", tag="ro")
                    nc.scalar.copy(ro[:], pr[:])
                    nc.scalar.dma_start(c2_in[P * jb:P * (jb + 1), :], ro[:])

            # ---- ReduceScatter readout partials (group-4) ----
            nc.gpsimd.collective_compute(
                "ReduceScatter", Alu.add, replica_groups=GROUPS4,
                ins=[c2_in.opt()], outs=[c2_out.opt()])

            # ---- final: relu(partial * deginv1), own 512 rows only ----
            fo = pers.tile([P, TI * 65], f32, name="fo")
            for j in range(TI):
                nc.scalar.dma_start(fo[:, j * 65:(j + 1) * 65],
                                    c2_out[P * j:P * (j + 1), :])
            dinv1 = pers.tile([P, TI], f32, name="dinv1")
            nc.vector.tensor_scalar(
                dinv1[:], fo[:, 64::65], float(EPS), None, op0=Alu.add)
            nc.vector.reciprocal(dinv1[:], dinv1[:])
            for j in range(TI):
                oj = work.tile([P, 64], f32, name="oj", tag="oj")
                nc.scalar.activation(oj[:], fo[:, j * 65:j * 65 + 64],
                                     Act.Relu, scale=dinv1[:, j:j + 1])
                nc.scalar.dma_start(out[P * j:P * (j + 1), :], oj[:])

    nc.finalize()
    return nc


def _get_nc():
    global _nc_cache
    if _nc_cache is None:
        _nc_cache = _build_nc()
    return _nc_cache


def _softmax(w):
    m = w.max(axis=1, keepdims=True)
    e = np.exp(w - m)
    return e / e.sum(axis=1, keepdims=True)


def _run(A, X, conv_w_l0_1, conv_w_l0_2, conv_w_l1, gcn_weight, trace=False):
    _install_ntff_hook()
    from concourse.bass_utils import run_bass_kernel_spmd

    bf16 = ml_dtypes.bfloat16
    A = np.ascontiguousarray(np.asarray(A, np.float32)).astype(bf16)
    X = np.asarray(X, np.float32)
    s1 = _softmax(np.asarray(conv_w_l0_1, np.float32)[:, :, 0, 0])  # [2, 5]
    s2 = _softmax(np.asarray(conv_w_l0_2, np.float32)[:, :, 0, 0])
    s3 = _softmax(np.asarray(conv_w_l1, np.float32)[:, :, 0, 0])
    w = np.ascontiguousarray(np.asarray(gcn_weight, np.float32)).astype(bf16)

    in_maps = []
    for c in range(8):
        r, g = c % 4, c // 4
        rows = slice(S * r, S * (r + 1))
        in_maps.append({
            "a_rows": np.ascontiguousarray(A[:, rows, :]),
            "s1": np.ascontiguousarray(np.broadcast_to(s1[g], (P, E))).astype(np.float32),
            "s2": np.ascontiguousarray(np.broadcast_to(s2[g], (P, E))).astype(np.float32),
            "s3": np.ascontiguousarray(np.broadcast_to(s3[g], (P, E))).astype(np.float32),
            "xt": np.ascontiguousarray(X[rows, :].T.astype(bf16)),
            "w": w,
            "doff": np.ascontiguousarray(np.broadcast_to(
                (S * r + P * np.arange(4, dtype=np.float32))[None, :],
                (P, 4))).astype(np.float32),
        })

    nc = _get_nc()
    res = run_bass_kernel_spmd(nc, in_maps, core_ids=list(range(8)), trace=trace)
    full = np.empty((N, 128), np.float32)
    for c in range(8):
        r, g = c % 4, c // 4
        full[S * r:S * (r + 1), 64 * g:64 * (g + 1)] = res.results[c]["out"]
    return np.ascontiguousarray(full), res


def kernel(A, X, conv_w_l0_1, conv_w_l0_2, conv_w_l1, gcn_weight):
    out, _ = _run(A, X, conv_w_l0_1, conv_w_l0_2, conv_w_l1, gcn_weight)
    return out


# revision 14
# speedup vs baseline: 1.8068x; 1.2260x over previous
"""GTN (Graph Transformer Network) kernel on 8 TRN2 NeuronCores via Bass/Tile.

Problem nn_GTN_17162689314910:
  A: [E=5, N=2048, N] f32, X: [N, 256] f32, conv_w_*: [C=2, E, 1, 1] f32,
  gcn_weight: [256, 64] f32 -> out [N, C*64] f32.

Math (per channel c):
  a = sum_e softmax(w1)[c,e] A[e];  b, a1 likewise with w2, w3
  H0 = a @ b
  H0n = H0 * 1/(colsum(H0)+eps)          (norm add=False; diag term dropped)
  H1 = H0n @ a1
  H1d = H1 with diag set to 1
  out_c = relu(H1d^T @ (X @ W) * 1/(colsum(H1d)+eps)[:,None])

Sharding: channel-split. Cores 0-3 = channel 0, cores 4-7 = channel 1;
within a group, 512-row shards. bf16 compute, fp32 PSUM.

v2 schedule (single A pass, consolidated collectives, H0T formulation):
  - One streaming pass over A computes all three convs: conv_b on PE
    (5 accumulating matmuls with diag(s_e) stationary), conv_a split
    DVE/PE, conv_a1 on DVE.
  - Two 8-core AllGathers (b then a1), 2MB/rank each, Shared outputs.
  - mm1 computes H0T = (a@b)^T directly: lhsT = gathered b panels
    (natural layout), rhs = aT. No second transpose round: H0T is
    exactly mm2's lhsT, and mm2's output H1 is natural for readout.
  - deg0 = colsum(H0) = free-dim rowsum of H0T (DVE) + 8KB group-4
    AllReduce; reciprocal on [128,16] layout (not [1,2048]).
  - readout partials ReduceScatter'd (group-4); each core emits only
    its own 512-row strip of the output.
  - PSUM->SBUF copies ride the Scalar engine; DMA is split across the
    two HWDGE rings (sync: A/b-cache/a1-panels, scalar: small/comm).
"""
import sys
import types

import numpy as np
import ml_dtypes

P = 128
N = 2048
S = 512            # shard rows per core
E = 5
TK = N // P        # 16 k tiles
TI = S // P        # 4 i tiles
Q = 512            # mm column-quarter width
EPS = 1e-8
GROUPS8 = [[0, 1, 2, 3, 4, 5, 6, 7]]
GROUPS4 = [[0, 1, 2, 3], [4, 5, 6, 7]]

_nc_cache = None


def _install_ntff_hook():
    if "antenv.axon_hooks" in sys.modules:
        return
    try:
        from trn_agent_boot.trn_boot import _ntff_profile_via_ctypes
        hook = _ntff_profile_via_ctypes("/opt/axon/libaxon_pjrt.so")
    except Exception:
        hook = None
    mod = types.ModuleType("antenv.axon_hooks")
    mod.get_axon_ntff_profile_hook = lambda: hook
    mod.set_axon_ntff_profile_hook = lambda h: None
    sys.modules["antenv.axon_hooks"] = mod


def _build_nc():
    import concourse.mybir as mybir
    import concourse.tile as tile
    from concourse import bacc
    from concourse.bass import ds

    bf16 = mybir.dt.bfloat16
    fp8 = mybir.dt.float8e4
    f32 = mybir.dt.float32
    u8 = mybir.dt.uint8
    Alu = mybir.AluOpType
    Act = mybir.ActivationFunctionType

    nc = bacc.Bacc(None)
    nc.num_devices = 8

    a_rows = nc.dram_tensor("a_rows", [E, S, N], bf16, kind="ExternalInput")
    s1 = nc.dram_tensor("s1", [P, E], f32, kind="ExternalInput")
    s2 = nc.dram_tensor("s2", [P, E], f32, kind="ExternalInput")
    s3 = nc.dram_tensor("s3", [P, E], f32, kind="ExternalInput")
    xt = nc.dram_tensor("xt", [256, S], bf16, kind="ExternalInput")
    w_in = nc.dram_tensor("w", [256, 64], bf16, kind="ExternalInput")
    identin = nc.dram_tensor("identin", [P, P], bf16, kind="ExternalInput")
    onesin = nc.dram_tensor("onesin", [P, Q], bf16, kind="ExternalInput")
    masks_in = nc.dram_tensor("masks_in", [TI * P, N], u8, kind="ExternalInput")
    out = nc.dram_tensor("out", [S, 64], f32, kind="ExternalOutput")

    with tile.TileContext(nc) as tc:
        with (
            tc.tile_pool(name="pers", bufs=1) as pers,
            tc.tile_pool(name="work", bufs=2) as work,
            tc.tile_pool(name="big", bufs=4) as bigp,
            tc.tile_pool(name="pan", bufs=4) as panp,
            tc.tile_pool(name="ps", bufs=8, space="PSUM") as psp,
            tc.tile_pool(name="dram", bufs=1, space="DRAM") as dram,
        ):
            pid = nc.partition_id()
            g4 = (pid // 4) * (4 * S)     # row base of own group in AG outputs

            # ---- small constants (scalar HWDGE ring) ----
            s1_sb = pers.tile([P, E], f32, name="s1_sb")
            s2_sb = pers.tile([P, E], f32, name="s2_sb")
            s3_sb = pers.tile([P, E], f32, name="s3_sb")
            ident = pers.tile([P, P], bf16, name="ident")
            nc.scalar.dma_start(s1_sb[:], s1[:])
            nc.scalar.dma_start(s2_sb[:], s2[:])
            nc.scalar.dma_start(s3_sb[:], s3[:])
            nc.scalar.dma_start(ident[:], identin[:])
            xt_sb = [pers.tile([P, S], bf16, name=f"xt_{k}") for k in range(2)]
            w_sb = [pers.tile([P, 64], bf16, name=f"w_{k}") for k in range(2)]
            for k in range(2):
                nc.scalar.dma_start(xt_sb[k][:], xt[P * k:P * (k + 1), :])
                nc.scalar.dma_start(w_sb[k][:], w_in[P * k:P * (k + 1), :])

            # diag(s_j[e]) stationary tiles for PE conv
            dscb = [pers.tile([P, P], bf16, name=f"dscb_{e}") for e in range(E)]
            dsca = [pers.tile([P, P], bf16, name=f"dsca_{e}") for e in range(E)]
            for e in range(E):
                nc.vector.tensor_scalar(
                    dscb[e][:], ident[:], s2_sb[:, e:e + 1], None, op0=Alu.mult)
                nc.vector.tensor_scalar(
                    dsca[e][:], ident[:], s1_sb[:, e:e + 1], None, op0=Alu.mult)

            # ---- collective DRAM buffers ----
            # b gathered in two row-chunks so the first collective can
            # launch after only 2 conv tiles (and mm1 can start on half
            # the k-blocks while chunk 2 is in flight). Payloads in fp8
            # to halve collective bytes; they feed matmuls directly.
            agb1_in = dram.tile([2 * P, N], fp8, name="agb1_in")
            agb2_in = dram.tile([2 * P, N], fp8, name="agb2_in")
            agb1_out = dram.tile([16 * P, N], fp8, name="agb1_out",
                                 addr_space="Shared")
            agb2_out = dram.tile([16 * P, N], fp8, name="agb2_out",
                                 addr_space="Shared")
            aga_in = dram.tile([S, N], fp8, name="aga_in")
            aga_out = dram.tile([8 * S, N], fp8, name="aga_out",
                                addr_space="Shared")
            deg_in1 = dram.tile([P, 8], f32, name="deg_in1")
            deg_out1 = dram.tile([P, 8], f32, name="deg_out1")
            deg_in2 = dram.tile([P, 8], f32, name="deg_in2")
            deg_out2 = dram.tile([P, 8], f32, name="deg_out2")
            c2_in = dram.tile([N, 65], f32, name="c2_in")
            c2_out = dram.tile([S, 65], f32, name="c2_out")

            # ---- A tiles: 4 row-tiles x 5 channels, single load ----
            At = [bigp.tile([P, E * N], bf16, name="At", tag="big")
                  for _ in range(TI)]
            for t in range(TI):
                for e in range(E):
                    nc.sync.dma_start(At[t][:, e * N:(e + 1) * N],
                                      a_rows[e, P * t:P * (t + 1), :])

            def conv_pe(dst_sb, t, dscs):
                # dst = sum_e s_e * A[e] via accumulating matmuls,
                # lhsT = diag(s_e) stationary, rhs = A tile quarters.
                for q in range(4):
                    cv = psp.tile([P, Q], f32, name="cv", tag="ps")
                    for e in range(E):
                        nc.tensor.matmul(
                            cv[:], dscs[e][:],
                            At[t][:, e * N + Q * q:e * N + Q * (q + 1)],
                            start=(e == 0), stop=(e == E - 1))
                    nc.scalar.copy(dst_sb[:, Q * q:Q * (q + 1)], cv[:])

            def conv_dve(dst_sb, t, s_ap):
                nc.vector.tensor_scalar(
                    dst_sb[:], At[t][:, 0:N], s_ap[:, 0:1], None, op0=Alu.mult)
                for e in range(1, E):
                    nc.vector.scalar_tensor_tensor(
                        dst_sb[:], At[t][:, e * N:(e + 1) * N], s_ap[:, e:e + 1],
                        dst_sb[:], op0=Alu.mult, op1=Alu.add)

            # ---- conv_b on PE -> chunked AG_B ASAP (fp8 payload) ----
            for t in range(TI):
                bt = work.tile([P, N], fp8, name="bt", tag="bt")
                conv_pe(bt, t, dscb)
                dst = agb1_in if t < 2 else agb2_in
                nc.scalar.dma_start(dst[P * (t % 2):P * (t % 2 + 1), :], bt[:])
                if t == 1:
                    nc.gpsimd.collective_compute(
                        "AllGather", Alu.bypass, replica_groups=GROUPS8,
                        ins=[agb1_in.opt()], outs=[agb1_out.opt()])
                elif t == 3:
                    nc.gpsimd.collective_compute(
                        "AllGather", Alu.bypass, replica_groups=GROUPS8,
                        ins=[agb2_in.opt()], outs=[agb2_out.opt()])

            # ---- conv_a: DVE t0,t1 + PE t2,t3 ----
            a_sb = [pers.tile([P, N], bf16, name=f"a_sb{t}") for t in range(TI)]
            conv_dve(a_sb[0], 0, s1_sb)
            conv_dve(a_sb[1], 1, s1_sb)
            conv_pe(a_sb[2], 2, dsca)
            conv_pe(a_sb[3], 3, dsca)

            # ---- conv_a1 on DVE -> fp8 -> AG_A1 ----
            for t in range(TI):
                a1t = work.tile([P, N], bf16, name="a1t", tag="a1t")
                conv_dve(a1t, t, s3_sb)
                a1f = work.tile([P, N], fp8, name="a1f", tag="a1f")
                nc.scalar.copy(a1f[:], a1t[:])
                nc.scalar.dma_start(aga_in[P * t:P * (t + 1), :], a1f[:])
            nc.gpsimd.collective_compute(
                "AllGather", Alu.bypass, replica_groups=GROUPS8,
                ins=[aga_in.opt()], outs=[aga_out.opt()])

            # ---- aT via PE transposes ----
            aT = [pers.tile([P, S], bf16, name=f"aT_{k}") for k in range(TK)]
            for t in range(TI):
                for k in range(TK):
                    pt = psp.tile([P, P], bf16, name="pt", tag="ps")
                    nc.tensor.transpose(pt[:], a_sb[t][:, P * k:P * (k + 1)],
                                        ident[:])
                    nc.scalar.copy(aT[k][:, P * t:P * (t + 1)], pt[:])

            # ---- Xw = (X @ W)[own rows], col 64 = 1 (colsum rider) ----
            xwo = [pers.tile([P, 65], bf16, name=f"xwo_{t}") for t in range(TI)]
            for t in range(TI):
                nc.gpsimd.memset(xwo[t][:, 64:65], 1.0)
            for t in range(TI):
                px = psp.tile([P, 64], f32, name="px", tag="ps")
                for k in range(2):
                    nc.tensor.matmul(px[:], xt_sb[k][:, P * t:P * (t + 1)],
                                     w_sb[k][:], start=(k == 0), stop=(k == 1))
                nc.scalar.copy(xwo[t][:, 0:64], px[:])

            # ---- b cache from AG_B outputs (reuses A-pool slots) ----
            # slot h: h0 = chunk1 ranks 0,1 (k 0,1,4,5), h1 = chunk1
            # ranks 2,3 (k 8,9,12,13), h2/h3 likewise from chunk2.
            g4b = (pid // 4) * (8 * P)    # group row base in chunk outputs
            bcg = [bigp.tile([P, 4 * N], fp8, name="At", tag="big")
                   for _ in range(4)]
            for h in range(4):
                src = agb1_out if h < 2 else agb2_out
                for idx in range(4):
                    q = 2 * (h % 2) + idx // 2
                    jj = idx % 2
                    nc.sync.dma_start(
                        bcg[h][:, idx * N:(idx + 1) * N],
                        src[ds(g4b + 2 * P * q + P * jj, P), :])

            # diag-fix constants come from host inputs (keeps gpsimd free
            # of everything but collective triggers)
            ones_q = pers.tile([P, Q], bf16, name="ones_q")
            nc.scalar.dma_start(ones_q[:], onesin[:])
            masks = [pers.tile([P, N], u8, name=f"mask_{t}") for t in range(TI)]
            for t in range(TI):
                nc.sync.dma_start(masks[t][:], masks_in[P * t:P * (t + 1), :])

            def bc_sl(k, ib):
                q, j = k // 4, k % 4
                h = 2 * (j // 2) + q // 2
                idx = 2 * (q % 2) + j % 2
                return bcg[h][:, idx * N + P * ib:idx * N + P * (ib + 1)]

            KORDER = [4 * q + j for jh in (0, 2) for qh in (0, 2)
                      for q in (qh, qh + 1) for j in (jh, jh + 1)]

            # ---- mm1: H0T[ib] = sum_k b[kblk, ib]^T-contracted with aT[k] ----
            #      out tile ib holds H0T[128ib:128(ib+1), own 512 rows]
            h0T = [pers.tile([P, S], bf16, name=f"h0T_{k}") for k in range(TK)]
            deg_sbT = pers.tile([P, TK], f32, name="deg_sbT")
            dinvT = pers.tile([P, TK], f32, name="dinvT")
            for ci, chunk in enumerate((range(0, 8), range(8, 16))):
                h0ps = {ib: psp.tile([P, S], f32, name=f"h0ps{ib}", tag="ps")
                        for ib in chunk}
                for ki, k in enumerate(KORDER):
                    for ib in chunk:
                        nc.tensor.matmul(h0ps[ib][:], bc_sl(k, ib), aT[k][:],
                                         start=(ki == 0), stop=(ki == TK - 1))
                for ib in chunk:
                    nc.scalar.copy(h0T[ib][:], h0ps[ib][:])
                    nc.vector.tensor_reduce(
                        deg_sbT[:, ib:ib + 1], h0T[ib][:],
                        mybir.AxisListType.X, Alu.add)
                # ---- deg0 AllReduce half (4KB, group-4) -> dinv0 half,
                #      scale the H0T half (AR of half 1 hides under half-2
                #      compute; mm2 k 0-7 can start before AR of half 2) ----
                dgi = deg_in1 if ci == 0 else deg_in2
                dgo = deg_out1 if ci == 0 else deg_out2
                lo = 8 * ci
                nc.scalar.dma_start(dgi[:], deg_sbT[:, lo:lo + 8])
                nc.gpsimd.collective_compute(
                    "AllReduce", Alu.add, replica_groups=GROUPS4,
                    ins=[dgi.opt()], outs=[dgo.opt()])
                nc.scalar.dma_start(dinvT[:, lo:lo + 8], dgo[:])
                nc.vector.tensor_scalar(dinvT[:, lo:lo + 8], dinvT[:, lo:lo + 8],
                                        float(EPS), None, op0=Alu.add)
                nc.vector.reciprocal(dinvT[:, lo:lo + 8], dinvT[:, lo:lo + 8])
                for ib in chunk:
                    nc.vector.tensor_scalar(h0T[ib][:], h0T[ib][:],
                                            dinvT[:, ib:ib + 1], None,
                                            op0=Alu.mult)

            # ---- mm2 + diag fix + readout, pipelined per column-quarter ----
            H1 = [pers.tile([P, N], bf16, name=f"H1_{t}") for t in range(TI)]
            for q in range(4):
                pts = [psp.tile([P, Q], f32, name=f"pt2_{i}", tag="ps")
                       for i in range(TI)]
                for k in range(TK):
                    pan = panp.tile([P, Q], fp8, name="pan", tag="pan")
                    nc.sync.dma_start(pan[:],
                                      aga_out[ds(g4 + P * k, P), Q * q:Q * (q + 1)])
                    for i in range(TI):
                        nc.tensor.matmul(pts[i][:], h0T[k][:, P * i:P * (i + 1)],
                                         pan[:], start=(k == 0), stop=(k == TK - 1))
                for i in range(TI):
                    nc.scalar.copy(H1[i][:, Q * q:Q * (q + 1)], pts[i][:])
                    nc.vector.copy_predicated(H1[i][:, Q * q:Q * (q + 1)],
                                              masks[i][:, Q * q:Q * (q + 1)],
                                              ones_q[:])
                for jb in range(4 * q, 4 * q + 4):
                    pr = psp.tile([P, 65], f32, name="pr", tag="ps")
                    for i in range(TI):
                        nc.tensor.matmul(pr[:], H1[i][:, P * jb:P * (jb + 1)],
                                         xwo[i][:], start=(i == 0), stop=(i == TI - 1))
                    ro = work.tile([P, 65], f32, name="ro", tag="ro")
                    nc.scalar.copy(ro[:], pr[:])
                    nc.scalar.dma_start(c2_in[P * jb:P * (jb + 1), :], ro[:])

            # ---- ReduceScatter readout partials (group-4) ----
            nc.gpsimd.collective_compute(
                "ReduceScatter", Alu.add, replica_groups=GROUPS4,
                ins=[c2_in.opt()], outs=[c2_out.opt()])

            # ---- final: relu(partial * deginv1), own 512 rows only ----
            fo = pers.tile([P, TI * 65], f32, name="fo")
            for j in range(TI):
                nc.scalar.dma_start(fo[:, j * 65:(j + 1) * 65],
                                    c2_out[P * j:P * (j + 1), :])
            dinv1 = pers.tile([P, TI], f32, name="dinv1")
            nc.vector.tensor_scalar(
                dinv1[:], fo[:, 64::65], float(EPS), None, op0=Alu.add)
            nc.vector.reciprocal(dinv1[:], dinv1[:])
            for j in range(TI):
                oj = work.tile([P, 64], f32, name="oj", tag="oj")
                nc.scalar.activation(oj[:], fo[:, j * 65:j * 65 + 64],
                                     Act.Relu, scale=dinv1[:, j:j + 1])
                nc.scalar.dma_start(out[P * j:P * (j + 1), :], oj[:])

    nc.finalize()
    return nc


def _get_nc():
    global _nc_cache
    if _nc_cache is None:
        _nc_cache = _build_nc()
    return _nc_cache


def _softmax(w):
    m = w.max(axis=1, keepdims=True)
    e = np.exp(w - m)
    return e / e.sum(axis=1, keepdims=True)


def _run(A, X, conv_w_l0_1, conv_w_l0_2, conv_w_l1, gcn_weight, trace=False):
    _install_ntff_hook()
    from concourse.bass_utils import run_bass_kernel_spmd

    bf16 = ml_dtypes.bfloat16
    A = np.ascontiguousarray(np.asarray(A, np.float32)).astype(bf16)
    X = np.asarray(X, np.float32)
    s1 = _softmax(np.asarray(conv_w_l0_1, np.float32)[:, :, 0, 0])  # [2, 5]
    s2 = _softmax(np.asarray(conv_w_l0_2, np.float32)[:, :, 0, 0])
    s3 = _softmax(np.asarray(conv_w_l1, np.float32)[:, :, 0, 0])
    w = np.ascontiguousarray(np.asarray(gcn_weight, np.float32)).astype(bf16)

    ident_np = np.eye(P, dtype=np.float32).astype(bf16)
    ones_np = np.ones((P, Q), np.float32).astype(bf16)
    in_maps = []
    for c in range(8):
        r, g = c % 4, c // 4
        rows = slice(S * r, S * (r + 1))
        masks_np = np.zeros((TI * P, N), np.uint8)
        for t in range(TI):
            for p in range(P):
                masks_np[t * P + p, S * r + P * t + p] = 1
        in_maps.append({
            "a_rows": np.ascontiguousarray(A[:, rows, :]),
            "s1": np.ascontiguousarray(np.broadcast_to(s1[g], (P, E))).astype(np.float32),
            "s2": np.ascontiguousarray(np.broadcast_to(s2[g], (P, E))).astype(np.float32),
            "s3": np.ascontiguousarray(np.broadcast_to(s3[g], (P, E))).astype(np.float32),
            "xt": np.ascontiguousarray(X[rows, :].T.astype(bf16)),
            "w": w,
            "identin": ident_np,
            "onesin": ones_np,
            "masks_in": masks_np,
        })

    nc = _get_nc()
    res = run_bass_kernel_spmd(nc, in_maps, core_ids=list(range(8)), trace=trace)
    full = np.empty((N, 128), np.float32)
    for c in range(8):
        r, g = c % 4, c // 4
        full[S * r:S * (r + 1), 64 * g:64 * (g + 1)] = res.results[c]["out"]
    return np.ascontiguousarray(full), res


def kernel(A, X, conv_w_l0_1, conv_w_l0_2, conv_w_l1, gcn_weight):
    out, _ = _run(A, X, conv_w_l0_1, conv_w_l0_2, conv_w_l1, gcn_weight)
    return out
